# revision 24
# baseline (speedup 1.0000x reference)
"""DINet retrieval-knn kernel for 8 trn2 NeuronCores.

Math (see reference): for each query patch q (3x3xC neighborhood of Q),
find k* = argmax_k cos(K_patch_k, Q_patch_q) over all 4096 key patches,
output S = max cosine value, T = fold(V_patch_gather(k*)) / 9.

Device strategy (per sharding hint): data-parallel over batch B (=2),
sequence-parallel over Q columns (4 shards of 1024) -> 8 cores. Each core
computes its full [Lk=4096, Lq=1024] correlation block with the tensor
engine (contraction C*9=576 in fp32), and a fused
copy+max (tensor_tensor_reduce) plus max_index pass gives max/argmax over
the full K axis per query. Host does layout prep (unfold, l2-normalize)
and the final V-gather + fold.
"""

import sys

import numpy as np

for _p in ("/opt/trn_rl_repo", "/root/.axon_site/_ro/trn_rl_repo"):
    if _p not in sys.path:
        sys.path.append(_p)

import concourse.bass as bass
import concourse.mybir as mybir
from concourse import bacc, bass_utils
from concourse.tile import TileContext

B, C, H, W = 2, 64, 64, 64
L = H * W            # 4096
C9 = C * 9           # 576
NSHARD = 4           # Q-column shards per batch
LQ = L // NSHARD     # 1024 query columns per core
NCORES = 8
NQB = LQ // 128      # 8 query blocks of 128
NKT = L // 512       # 8 key column tiles of 512
CP = 640             # contraction padded to 5 x 128

EPS = 1e-12

_BASS_CACHE = {}


def _build_bass():
    f32 = mybir.dt.float32
    bf16 = mybir.dt.bfloat16  # full-rate PE + FWL weight loads + half DMA
    u32 = mybir.dt.uint32
    # Bacc (not plain Bass): its compile() runs move_matmul_waits_to_ldweights
    # + generate_event_semaphores, which split multi-wait instructions that
    # walrus otherwise rejects ("Too many sync wait commands")
    nc = bacc.Bacc("TRN2")

    # contraction padded 576 -> 640 on host: 5 uniform 128-row chunks so
    # every weight load is a full-128 FWL load
    kn = nc.dram_tensor("kn", [CP, L], bf16, kind="ExternalInput")
    qn = nc.dram_tensor("qn", [CP, LQ], bf16, kind="ExternalInput")
    # top-8 candidate indices per (query column, 2048-wide k-half); host
    # re-scores the 16 candidates exactly in fp32
    rarg = nc.dram_tensor("rarg", [128, NQB * 16], u32, kind="ExternalOutput")

    with TileContext(nc) as tc:
        with (
            tc.tile_pool(name="kpool", bufs=1) as kpool,
            tc.tile_pool(name="qpool", bufs=1) as qpool,
            tc.tile_pool(name="rpool", bufs=1) as rpool,
            tc.tile_pool(name="ppool", bufs=4, space="PSUM") as ppool,
            tc.tile_pool(name="mpool", bufs=4) as mpool,
            tc.tile_pool(name="opool", bufs=1) as opool,
        ):
            qt = [
                qpool.tile([128, LQ], bf16, name=f"qt{t}", tag=f"qt{t}")
                for t in range(5)
            ]
            kt = [
                [
                    kpool.tile([128, 2048], bf16, name=f"kt{t}_{kh}", tag=f"kt{t}_{kh}")
                    for kh in range(2)
                ]
                for t in range(5)
            ]
            # DMA issue order ~ consumption order
            for t in range(5):
                nc.sync.dma_start(out=qt[t], in_=qn[t * 128 : (t + 1) * 128, :])
            for kh in range(2):
                for t in range(5):
                    nc.sync.dma_start(
                        out=kt[t][kh],
                        in_=kn[t * 128 : (t + 1) * 128, kh * 2048 : (kh + 1) * 2048],
                    )

            # per-(qb, half) R tiles persist until their scan
            rt = [
                [
                    rpool.tile([128, 2048], bf16, name=f"rt{qb}_{kh}", tag=f"rt{qb}_{kh}")
                    for kh in range(2)
                ]
                for qb in range(NQB)
            ]

            def group10(qb, npair, on_dve=False):
                """one 2-bank psum group: k columns [npair*1024, +1024)"""
                kh, gg = divmod(npair, 2)
                ps = ppool.tile([128, 1024], f32, name="ps", tag="ps")
                for t in range(5):
                    for n in range(2):
                        nc.tensor.matmul(
                            ps[:, n * 512 : (n + 1) * 512],
                            lhsT=qt[t][:, qb * 128 : (qb + 1) * 128],
                            rhs=kt[t][kh][
                                :, (gg * 2 + n) * 512 : (gg * 2 + n + 1) * 512
                            ],
                            start=(t == 0),
                            stop=(t == 4),
                        )
                # downcast to bf16: halves the DVE scan cost (2x mode).
                # on_dve: keep the whole tail chain on one engine (fewer
                # cross-engine semaphore hops) for the critical last block
                eng = nc.vector if on_dve else nc.scalar
                if on_dve:
                    eng.tensor_copy(
                        out=rt[qb][kh][:, gg * 1024 : (gg + 1) * 1024], in_=ps
                    )
                else:
                    eng.copy(out=rt[qb][kh][:, gg * 1024 : (gg + 1) * 1024], in_=ps)

            def scan(qb, kh):
                g = qb * 2 + kh
                mx8 = mpool.tile([128, 8], bf16, name="mx8", tag="mx8")
                nc.vector.max(out=mx8, in_=rt[qb][kh])
                # distinct output tile + own DMA per scan: avoids the serial
                # WAW chain a shared output tile would create
                idx8 = opool.tile([128, 8], u32, name=f"idx{g}", tag=f"idx{g}")
                nc.vector.max_index(out=idx8, in_max=mx8, in_values=rt[qb][kh])
                nc.sync.dma_start(out=rarg[:, g * 8 : (g + 1) * 8], in_=idx8)

            # staircase schedule: k-slice-major through npair 2 (keeps the
            # PE dense and the prologue short), query-major endgame so the
            # second-half scans stagger instead of bunching after the last
            # matmul
            for npair in range(2):
                for qb in range(NQB):
                    group10(qb, npair)
                    if npair == 1:
                        scan(qb, 0)
            for qb in range(NQB):
                group10(qb, 2)
            for qb in range(NQB):
                group10(qb, 3, on_dve=(qb == NQB - 1))
                scan(qb, 1)
    if not nc.is_finalized():
        nc.finalize()
    return nc


def _unfold_ij(x):
    """[B,C,H,W] -> [B, 9*C, H*W] with row = ij*C + c (ij-major order)."""
    b, c, h, w = x.shape
    xp = np.pad(x, ((0, 0), (0, 0), (1, 1), (1, 1)))
    blocks = [
        xp[:, :, i : i + h, j : j + w].reshape(b, c, h * w)
        for i in range(3)
        for j in range(3)
    ]
    return np.concatenate(blocks, axis=1)


def _unfold_torch(x):
    """[B,C,H,W] -> [B, C*9, H*W] in torch F.unfold order (c-major)."""
    b, c, h, w = x.shape
    xp = np.pad(x, ((0, 0), (0, 0), (1, 1), (1, 1)))
    patches = np.stack(
        [xp[:, :, i : i + h, j : j + w] for i in range(3) for j in range(3)],
        axis=2,
    )
    return patches.reshape(b, c * 9, h * w)


def _fold_torch(u, h, w):
    """Inverse layout of _unfold_torch: sum overlapping patches."""
    b, ck, l = u.shape
    c = ck // 9
    p = u.reshape(b, c, 3, 3, h, w)
    out = np.zeros((b, c, h + 2, w + 2), u.dtype)
    for i in range(3):
        for j in range(3):
            out[:, :, i : i + h, j : j + w] += p[:, :, i, j]
    return out[:, :, 1 : 1 + h, 1 : 1 + w]


def _l2n_cols(x):
    """Normalize columns of [B, C9, L] (fp32, eps as in reference)."""
    n = np.sqrt(np.sum(x * x, axis=1, keepdims=True, dtype=np.float32))
    return x / np.maximum(n, EPS)


def _run_device(Kn, Qn, trace=False, trace_cores=None):
    import ml_dtypes

    key = "nc"
    if key not in _BASS_CACHE:
        _BASS_CACHE[key] = _build_bass()
    nc = _BASS_CACHE[key]
    bf = ml_dtypes.bfloat16
    in_maps = []
    for ci in range(NCORES):
        b, s = divmod(ci, NSHARD)
        pad = ((0, CP - C9), (0, 0))
        in_maps.append(
            {
                "kn": np.ascontiguousarray(np.pad(Kn[b], pad).astype(bf)),
                "qn": np.ascontiguousarray(
                    np.pad(Qn[b][:, s * LQ : (s + 1) * LQ], pad).astype(bf)
                ),
            }
        )
    res = bass_utils.run_bass_kernel_spmd(
        nc,
        in_maps,
        core_ids=list(range(NCORES)),
        trace=trace,
        trace_cores=trace_cores,
    )
    return res


def kernel(V, K, Q, _trace=False, _trace_cores=None, _return_results=False):
    V = np.asarray(V, dtype=np.float32)
    K = np.asarray(K, dtype=np.float32)
    Q = np.asarray(Q, dtype=np.float32)

    Kn = _l2n_cols(_unfold_ij(K))
    Qn = _l2n_cols(_unfold_ij(Q))

    res = _run_device(Kn, Qn, trace=_trace, trace_cores=_trace_cores)

    # device returns top-8 candidate k per (query, 2048-wide k-half) under
    # bf16 matmul scores; re-score the 16 candidates exactly in fp32
    NC = 16
    cand = np.empty((B, L, NC), np.int64)
    for ci in range(NCORES):
        b, s = divmod(ci, NSHARD)
        out = np.asarray(res.results[ci]["rarg"]).astype(np.int64)
        # out[p, (qb*2+kh)*8+j] -> local q index qb*128 + p, k = kh*2048 + idx
        c = out.reshape(128, NQB, 2, 8)
        c = np.clip(c, 0, 2047) + np.arange(2)[None, None, :, None] * 2048
        cand[b, s * LQ : (s + 1) * LQ] = (
            c.reshape(128, NQB, NC).transpose(1, 0, 2).reshape(LQ, NC)
        )

    rstar = np.empty((B, L), np.float32)
    rarg = np.empty((B, L), np.int64)
    for b in range(B):
        kc = Kn[b][:, cand[b].reshape(-1)].reshape(C9, L, NC)
        scores = np.einsum("cqj,cq->qj", kc, Qn[b], dtype=np.float64)
        maxv = scores.max(axis=1, keepdims=True)
        kmask = np.where(scores == maxv, cand[b], 1 << 40)
        rarg[b] = kmask.min(axis=1)  # first occurrence on ties, like argmax
        rstar[b] = maxv[:, 0].astype(np.float32)

    V_unf = _unfold_torch(V)
    T_unf = np.take_along_axis(V_unf, rarg[:, None, :], axis=2)
    T = (_fold_torch(T_unf, H, W) / 9.0).astype(np.float32)
    S = rstar.reshape(B, 1, H, W)

    if _return_results:
        return (S, T), res
    return (S, T)


# revision 25
# speedup vs baseline: 1.1661x; 1.1661x over previous
"""DINet retrieval-knn kernel for 8 trn2 NeuronCores.

Math (see reference): for each query patch q (3x3xC neighborhood of Q),
find k* = argmax_k cos(K_patch_k, Q_patch_q) over all 4096 key patches,
output S = max cosine value, T = fold(V_patch_gather(k*)) / 9.

Device strategy (per sharding hint): data-parallel over batch B (=2),
sequence-parallel over Q columns (4 shards of 1024) -> 8 cores. Each core
computes its full [Lk=4096, Lq=1024] correlation block with the tensor
engine (contraction C*9=576 in fp32), and a fused
copy+max (tensor_tensor_reduce) plus max_index pass gives max/argmax over
the full K axis per query. Host does layout prep (unfold, l2-normalize)
and the final V-gather + fold.
"""

import sys

import numpy as np

for _p in ("/opt/trn_rl_repo", "/root/.axon_site/_ro/trn_rl_repo"):
    if _p not in sys.path:
        sys.path.append(_p)

import concourse.bass as bass
import concourse.mybir as mybir
from concourse import bacc, bass_utils
from concourse.tile import TileContext

B, C, H, W = 2, 64, 64, 64
L = H * W            # 4096
C9 = C * 9           # 576
NSHARD = 4           # Q-column shards per batch
LQ = L // NSHARD     # 1024 query columns per core
NCORES = 8
NQB = LQ // 128      # 8 query blocks of 128
NKT = L // 512       # 8 key column tiles of 512
CP = 640             # contraction padded to 5 x 128

EPS = 1e-12

_BASS_CACHE = {}


def _build_bass():
    f32 = mybir.dt.float32
    bf16 = mybir.dt.bfloat16  # full-rate PE + FWL weight loads + half DMA
    u32 = mybir.dt.uint32
    # Bacc (not plain Bass): its compile() runs move_matmul_waits_to_ldweights
    # + generate_event_semaphores, which split multi-wait instructions that
    # walrus otherwise rejects ("Too many sync wait commands")
    nc = bacc.Bacc("TRN2")

    # contraction padded 576 -> 640 on host: 5 uniform 128-row chunks so
    # every weight load is a full-128 FWL load
    kn = nc.dram_tensor("kn", [CP, L], bf16, kind="ExternalInput")
    qn = nc.dram_tensor("qn", [CP, LQ], bf16, kind="ExternalInput")
    # top-8 candidate indices per (query column, 2048-wide k-half); host
    # re-scores the 16 candidates exactly in fp32
    rarg = nc.dram_tensor("rarg", [128, NQB * 16], u32, kind="ExternalOutput")

    with TileContext(nc) as tc:
        with (
            tc.tile_pool(name="kpool", bufs=1) as kpool,
            tc.tile_pool(name="qpool", bufs=1) as qpool,
            tc.tile_pool(name="rpool", bufs=1) as rpool,
            tc.tile_pool(name="ppool", bufs=4, space="PSUM") as ppool,
            tc.tile_pool(name="mpool", bufs=4) as mpool,
            tc.tile_pool(name="opool", bufs=1) as opool,
        ):
            qt = [
                qpool.tile([128, LQ], bf16, name=f"qt{t}", tag=f"qt{t}")
                for t in range(5)
            ]
            kt = [
                [
                    kpool.tile([128, 2048], bf16, name=f"kt{t}_{kh}", tag=f"kt{t}_{kh}")
                    for kh in range(2)
                ]
                for t in range(5)
            ]
            # DMA issue order ~ consumption order
            for t in range(5):
                nc.sync.dma_start(out=qt[t], in_=qn[t * 128 : (t + 1) * 128, :])
            for kh in range(2):
                for t in range(5):
                    nc.sync.dma_start(
                        out=kt[t][kh],
                        in_=kn[t * 128 : (t + 1) * 128, kh * 2048 : (kh + 1) * 2048],
                    )

            # per-(qb, half) R tiles persist until their scan
            rt = [
                [
                    rpool.tile([128, 2048], bf16, name=f"rt{qb}_{kh}", tag=f"rt{qb}_{kh}")
                    for kh in range(2)
                ]
                for qb in range(NQB)
            ]

            def group10(qb, npair, on_dve=False):
                """one 2-bank psum group: k columns [npair*1024, +1024)"""
                kh, gg = divmod(npair, 2)
                ps = ppool.tile([128, 1024], f32, name="ps", tag="ps")
                for t in range(5):
                    for n in range(2):
                        nc.tensor.matmul(
                            ps[:, n * 512 : (n + 1) * 512],
                            lhsT=qt[t][:, qb * 128 : (qb + 1) * 128],
                            rhs=kt[t][kh][
                                :, (gg * 2 + n) * 512 : (gg * 2 + n + 1) * 512
                            ],
                            start=(t == 0),
                            stop=(t == 4),
                        )
                # downcast to bf16: halves the DVE scan cost (2x mode).
                # on_dve: keep the whole tail chain on one engine (fewer
                # cross-engine semaphore hops) for the critical last block
                eng = nc.vector if on_dve else nc.scalar
                if on_dve:
                    eng.tensor_copy(
                        out=rt[qb][kh][:, gg * 1024 : (gg + 1) * 1024], in_=ps
                    )
                else:
                    eng.copy(out=rt[qb][kh][:, gg * 1024 : (gg + 1) * 1024], in_=ps)

            def scan(qb, kh):
                """top-8 of the 4-to-1 max-reduced slice; each winning
                position j stands for original columns j + {0,512,1024,1536}
                (host expands + re-scores). bf16 tensor_max runs at 2x, so
                the reduction tree halves the DVE cost vs scanning 2048."""
                g = qb * 2 + kh
                m1 = mpool.tile([128, 1024], bf16, name="m1", tag="m1")
                nc.vector.tensor_max(m1, rt[qb][kh][:, 0:1024], rt[qb][kh][:, 1024:2048])
                m2 = mpool.tile([128, 512], bf16, name="m2", tag="m2")
                nc.vector.tensor_max(m2, m1[:, 0:512], m1[:, 512:1024])
                mx8 = mpool.tile([128, 8], bf16, name="mx8", tag="mx8")
                nc.vector.max(out=mx8, in_=m2)
                # distinct output tile + own DMA per scan: avoids the serial
                # WAW chain a shared output tile would create
                idx8 = opool.tile([128, 8], u32, name=f"idx{g}", tag=f"idx{g}")
                nc.vector.max_index(out=idx8, in_max=mx8, in_values=m2)
                nc.sync.dma_start(out=rarg[:, g * 8 : (g + 1) * 8], in_=idx8)

            # staircase schedule: k-slice-major through npair 2 (keeps the
            # PE dense and the prologue short), query-major endgame so the
            # second-half scans stagger instead of bunching after the last
            # matmul
            for npair in range(2):
                for qb in range(NQB):
                    group10(qb, npair)
                    if npair == 1:
                        scan(qb, 0)
            for qb in range(NQB):
                group10(qb, 2)
            for qb in range(NQB):
                group10(qb, 3, on_dve=(qb == NQB - 1))
                scan(qb, 1)
    if not nc.is_finalized():
        nc.finalize()
    return nc


def _unfold_ij(x):
    """[B,C,H,W] -> [B, 9*C, H*W] with row = ij*C + c (ij-major order)."""
    b, c, h, w = x.shape
    xp = np.pad(x, ((0, 0), (0, 0), (1, 1), (1, 1)))
    blocks = [
        xp[:, :, i : i + h, j : j + w].reshape(b, c, h * w)
        for i in range(3)
        for j in range(3)
    ]
    return np.concatenate(blocks, axis=1)


def _unfold_torch(x):
    """[B,C,H,W] -> [B, C*9, H*W] in torch F.unfold order (c-major)."""
    b, c, h, w = x.shape
    xp = np.pad(x, ((0, 0), (0, 0), (1, 1), (1, 1)))
    patches = np.stack(
        [xp[:, :, i : i + h, j : j + w] for i in range(3) for j in range(3)],
        axis=2,
    )
    return patches.reshape(b, c * 9, h * w)


def _fold_torch(u, h, w):
    """Inverse layout of _unfold_torch: sum overlapping patches."""
    b, ck, l = u.shape
    c = ck // 9
    p = u.reshape(b, c, 3, 3, h, w)
    out = np.zeros((b, c, h + 2, w + 2), u.dtype)
    for i in range(3):
        for j in range(3):
            out[:, :, i : i + h, j : j + w] += p[:, :, i, j]
    return out[:, :, 1 : 1 + h, 1 : 1 + w]


def _l2n_cols(x):
    """Normalize columns of [B, C9, L] (fp32, eps as in reference)."""
    n = np.sqrt(np.sum(x * x, axis=1, keepdims=True, dtype=np.float32))
    return x / np.maximum(n, EPS)


def _run_device(Kn, Qn, trace=False, trace_cores=None):
    import ml_dtypes

    key = "nc"
    if key not in _BASS_CACHE:
        _BASS_CACHE[key] = _build_bass()
    nc = _BASS_CACHE[key]
    bf = ml_dtypes.bfloat16
    in_maps = []
    for ci in range(NCORES):
        b, s = divmod(ci, NSHARD)
        pad = ((0, CP - C9), (0, 0))
        in_maps.append(
            {
                "kn": np.ascontiguousarray(np.pad(Kn[b], pad).astype(bf)),
                "qn": np.ascontiguousarray(
                    np.pad(Qn[b][:, s * LQ : (s + 1) * LQ], pad).astype(bf)
                ),
            }
        )
    res = bass_utils.run_bass_kernel_spmd(
        nc,
        in_maps,
        core_ids=list(range(NCORES)),
        trace=trace,
        trace_cores=trace_cores,
    )
    return res


def kernel(V, K, Q, _trace=False, _trace_cores=None, _return_results=False):
    V = np.asarray(V, dtype=np.float32)
    K = np.asarray(K, dtype=np.float32)
    Q = np.asarray(Q, dtype=np.float32)

    Kn = _l2n_cols(_unfold_ij(K))
    Qn = _l2n_cols(_unfold_ij(Q))

    res = _run_device(Kn, Qn, trace=_trace, trace_cores=_trace_cores)

    # device returns, per (query, 2048-wide k-half), the top-8 positions of
    # the 4->1 max-reduced slice under bf16 scores; expand each position to
    # its 4 source columns and re-score the 64 candidates exactly in fp32
    NC = 64
    cand = np.empty((B, L, NC), np.int64)
    for ci in range(NCORES):
        b, s = divmod(ci, NSHARD)
        out = np.asarray(res.results[ci]["rarg"]).astype(np.int64)
        # out[p, (qb*2+kh)*8+j] in [0,512) -> k = kh*2048 + j + {0,512,1024,1536}
        c = out.reshape(128, NQB, 2, 8)
        c = np.clip(c, 0, 511)
        c = (
            c[..., None]
            + np.arange(4)[None, None, None, None, :] * 512
            + np.arange(2)[None, None, :, None, None] * 2048
        )  # [128, NQB, 2, 8, 4]
        cand[b, s * LQ : (s + 1) * LQ] = (
            c.reshape(128, NQB, NC).transpose(1, 0, 2).reshape(LQ, NC)
        )

    rstar = np.empty((B, L), np.float32)
    rarg = np.empty((B, L), np.int64)
    QCH = 512
    for b in range(B):
        for q0 in range(0, L, QCH):
            cb = cand[b, q0 : q0 + QCH]
            kc = Kn[b][:, cb.reshape(-1)].reshape(C9, QCH, NC)
            scores = np.einsum("cqj,cq->qj", kc, Qn[b][:, q0 : q0 + QCH],
                               dtype=np.float64)
            maxv = scores.max(axis=1, keepdims=True)
            kmask = np.where(scores == maxv, cb, 1 << 40)
            rarg[b, q0 : q0 + QCH] = kmask.min(axis=1)  # first occurrence on ties
            rstar[b, q0 : q0 + QCH] = maxv[:, 0].astype(np.float32)

    V_unf = _unfold_torch(V)
    T_unf = np.take_along_axis(V_unf, rarg[:, None, :], axis=2)
    T = (_fold_torch(T_unf, H, W) / 9.0).astype(np.float32)
    S = rstar.reshape(B, 1, H, W)

    if _return_results:
        return (S, T), res
    return (S, T)


# revision 26
# speedup vs baseline: 1.2016x; 1.0304x over previous
"""DINet retrieval-knn kernel for 8 trn2 NeuronCores.

Math (see reference): for each query patch q (3x3xC neighborhood of Q),
find k* = argmax_k cos(K_patch_k, Q_patch_q) over all 4096 key patches,
output S = max cosine value, T = fold(V_patch_gather(k*)) / 9.

Device strategy (per sharding hint): data-parallel over batch B (=2),
sequence-parallel over Q columns (4 shards of 1024) -> 8 cores. Each core
computes its full [Lk=4096, Lq=1024] correlation block with the tensor
engine (contraction C*9=576 in fp32), and a fused
copy+max (tensor_tensor_reduce) plus max_index pass gives max/argmax over
the full K axis per query. Host does layout prep (unfold, l2-normalize)
and the final V-gather + fold.
"""

import sys

import numpy as np

for _p in ("/opt/trn_rl_repo", "/root/.axon_site/_ro/trn_rl_repo"):
    if _p not in sys.path:
        sys.path.append(_p)

import concourse.bass as bass
import concourse.mybir as mybir
from concourse import bacc, bass_utils
from concourse.tile import TileContext

B, C, H, W = 2, 64, 64, 64
L = H * W            # 4096
C9 = C * 9           # 576
NSHARD = 4           # Q-column shards per batch
LQ = L // NSHARD     # 1024 query columns per core
NCORES = 8
NQB = LQ // 128      # 8 query blocks of 128
NKT = L // 512       # 8 key column tiles of 512
CP = 640             # contraction padded to 5 x 128

EPS = 1e-12

_BASS_CACHE = {}


def _build_bass():
    f32 = mybir.dt.float32
    bf16 = mybir.dt.bfloat16  # full-rate PE + FWL weight loads + half DMA
    u32 = mybir.dt.uint32
    # Bacc (not plain Bass): its compile() runs move_matmul_waits_to_ldweights
    # + generate_event_semaphores, which split multi-wait instructions that
    # walrus otherwise rejects ("Too many sync wait commands")
    nc = bacc.Bacc("TRN2")

    # contraction padded 576 -> 640 on host: 5 uniform 128-row chunks so
    # every weight load is a full-128 FWL load
    kn = nc.dram_tensor("kn", [CP, L], bf16, kind="ExternalInput")
    qn = nc.dram_tensor("qn", [CP, LQ], bf16, kind="ExternalInput")
    # top-8 candidate indices per (query column, 2048-wide k-half); host
    # re-scores the 16 candidates exactly in fp32
    rarg = nc.dram_tensor("rarg", [128, NQB * 16], u32, kind="ExternalOutput")

    with TileContext(nc) as tc:
        with (
            tc.tile_pool(name="kpool", bufs=1) as kpool,
            tc.tile_pool(name="qpool", bufs=1) as qpool,
            tc.tile_pool(name="rpool", bufs=1) as rpool,
            tc.tile_pool(name="ppool", bufs=4, space="PSUM") as ppool,
            tc.tile_pool(name="mpool", bufs=4) as mpool,
            tc.tile_pool(name="opool", bufs=1) as opool,
        ):
            qt = [
                qpool.tile([128, LQ], bf16, name=f"qt{t}", tag=f"qt{t}")
                for t in range(5)
            ]
            kt = [
                [
                    kpool.tile([128, 1024], bf16, name=f"kt{t}_{np}", tag=f"kt{t}_{np}")
                    for np in range(4)
                ]
                for t in range(5)
            ]
            # DMA issue order ~ consumption order
            for t in range(5):
                nc.sync.dma_start(out=qt[t], in_=qn[t * 128 : (t + 1) * 128, :])
            for np in range(4):
                for t in range(5):
                    nc.sync.dma_start(
                        out=kt[t][np],
                        in_=kn[t * 128 : (t + 1) * 128, np * 1024 : (np + 1) * 1024],
                    )

            # per-(qb, half) R tiles persist until their scan
            rt = [
                [
                    rpool.tile([128, 2048], bf16, name=f"rt{qb}_{kh}", tag=f"rt{qb}_{kh}")
                    for kh in range(2)
                ]
                for qb in range(NQB)
            ]

            def group10(qb, npair, on_dve=False):
                """one 2-bank psum group: k columns [npair*1024, +1024)"""
                kh, gg = divmod(npair, 2)
                ps = ppool.tile([128, 1024], f32, name="ps", tag="ps")
                for t in range(5):
                    for n in range(2):
                        nc.tensor.matmul(
                            ps[:, n * 512 : (n + 1) * 512],
                            lhsT=qt[t][:, qb * 128 : (qb + 1) * 128],
                            rhs=kt[t][npair][:, n * 512 : (n + 1) * 512],
                            start=(t == 0),
                            stop=(t == 4),
                        )
                # downcast to bf16: halves the DVE scan cost (2x mode).
                # on_dve: keep the whole tail chain on one engine (fewer
                # cross-engine semaphore hops) for the critical last block
                eng = nc.vector if on_dve else nc.scalar
                if on_dve:
                    eng.tensor_copy(
                        out=rt[qb][kh][:, gg * 1024 : (gg + 1) * 1024], in_=ps
                    )
                else:
                    eng.copy(out=rt[qb][kh][:, gg * 1024 : (gg + 1) * 1024], in_=ps)

            def scan(qb, kh):
                """top-8 of the 4-to-1 max-reduced slice; each winning
                position j stands for original columns j + {0,512,1024,1536}
                (host expands + re-scores). bf16 tensor_max runs at 2x, so
                the reduction tree halves the DVE cost vs scanning 2048."""
                g = qb * 2 + kh
                m1 = mpool.tile([128, 1024], bf16, name="m1", tag="m1")
                nc.vector.tensor_max(m1, rt[qb][kh][:, 0:1024], rt[qb][kh][:, 1024:2048])
                m2 = mpool.tile([128, 512], bf16, name="m2", tag="m2")
                nc.vector.tensor_max(m2, m1[:, 0:512], m1[:, 512:1024])
                mx8 = mpool.tile([128, 8], bf16, name="mx8", tag="mx8")
                nc.vector.max(out=mx8, in_=m2)
                # distinct output tile + own DMA per scan: avoids the serial
                # WAW chain a shared output tile would create
                idx8 = opool.tile([128, 8], u32, name=f"idx{g}", tag=f"idx{g}")
                nc.vector.max_index(out=idx8, in_max=mx8, in_values=m2)
                nc.sync.dma_start(out=rarg[:, g * 8 : (g + 1) * 8], in_=idx8)

            # staircase schedule: k-slice-major through npair 2 (keeps the
            # PE dense and the prologue short), query-major endgame so the
            # second-half scans stagger instead of bunching after the last
            # matmul
            for npair in range(2):
                for qb in range(NQB):
                    group10(qb, npair)
                    if npair == 1:
                        scan(qb, 0)
            for qb in range(NQB):
                group10(qb, 2)
            for qb in range(NQB):
                group10(qb, 3, on_dve=(qb == NQB - 1))
                scan(qb, 1)
    if not nc.is_finalized():
        nc.finalize()
    return nc


def _unfold_ij(x):
    """[B,C,H,W] -> [B, 9*C, H*W] with row = ij*C + c (ij-major order)."""
    b, c, h, w = x.shape
    xp = np.pad(x, ((0, 0), (0, 0), (1, 1), (1, 1)))
    blocks = [
        xp[:, :, i : i + h, j : j + w].reshape(b, c, h * w)
        for i in range(3)
        for j in range(3)
    ]
    return np.concatenate(blocks, axis=1)


def _unfold_torch(x):
    """[B,C,H,W] -> [B, C*9, H*W] in torch F.unfold order (c-major)."""
    b, c, h, w = x.shape
    xp = np.pad(x, ((0, 0), (0, 0), (1, 1), (1, 1)))
    patches = np.stack(
        [xp[:, :, i : i + h, j : j + w] for i in range(3) for j in range(3)],
        axis=2,
    )
    return patches.reshape(b, c * 9, h * w)


def _fold_torch(u, h, w):
    """Inverse layout of _unfold_torch: sum overlapping patches."""
    b, ck, l = u.shape
    c = ck // 9
    p = u.reshape(b, c, 3, 3, h, w)
    out = np.zeros((b, c, h + 2, w + 2), u.dtype)
    for i in range(3):
        for j in range(3):
            out[:, :, i : i + h, j : j + w] += p[:, :, i, j]
    return out[:, :, 1 : 1 + h, 1 : 1 + w]


def _l2n_cols(x):
    """Normalize columns of [B, C9, L] (fp32, eps as in reference)."""
    n = np.sqrt(np.sum(x * x, axis=1, keepdims=True, dtype=np.float32))
    return x / np.maximum(n, EPS)


def _run_device(Kn, Qn, trace=False, trace_cores=None):
    import ml_dtypes

    key = "nc"
    if key not in _BASS_CACHE:
        _BASS_CACHE[key] = _build_bass()
    nc = _BASS_CACHE[key]
    bf = ml_dtypes.bfloat16
    in_maps = []
    for ci in range(NCORES):
        b, s = divmod(ci, NSHARD)
        pad = ((0, CP - C9), (0, 0))
        in_maps.append(
            {
                "kn": np.ascontiguousarray(np.pad(Kn[b], pad).astype(bf)),
                "qn": np.ascontiguousarray(
                    np.pad(Qn[b][:, s * LQ : (s + 1) * LQ], pad).astype(bf)
                ),
            }
        )
    res = bass_utils.run_bass_kernel_spmd(
        nc,
        in_maps,
        core_ids=list(range(NCORES)),
        trace=trace,
        trace_cores=trace_cores,
    )
    return res


def kernel(V, K, Q, _trace=False, _trace_cores=None, _return_results=False):
    V = np.asarray(V, dtype=np.float32)
    K = np.asarray(K, dtype=np.float32)
    Q = np.asarray(Q, dtype=np.float32)

    Kn = _l2n_cols(_unfold_ij(K))
    Qn = _l2n_cols(_unfold_ij(Q))

    res = _run_device(Kn, Qn, trace=_trace, trace_cores=_trace_cores)

    # device returns, per (query, 2048-wide k-half), the top-8 positions of
    # the 4->1 max-reduced slice under bf16 scores; expand each position to
    # its 4 source columns and re-score the 64 candidates exactly in fp32
    NC = 64
    cand = np.empty((B, L, NC), np.int64)
    for ci in range(NCORES):
        b, s = divmod(ci, NSHARD)
        out = np.asarray(res.results[ci]["rarg"]).astype(np.int64)
        # out[p, (qb*2+kh)*8+j] in [0,512) -> k = kh*2048 + j + {0,512,1024,1536}
        c = out.reshape(128, NQB, 2, 8)
        c = np.clip(c, 0, 511)
        c = (
            c[..., None]
            + np.arange(4)[None, None, None, None, :] * 512
            + np.arange(2)[None, None, :, None, None] * 2048
        )  # [128, NQB, 2, 8, 4]
        cand[b, s * LQ : (s + 1) * LQ] = (
            c.reshape(128, NQB, NC).transpose(1, 0, 2).reshape(LQ, NC)
        )

    rstar = np.empty((B, L), np.float32)
    rarg = np.empty((B, L), np.int64)
    QCH = 512
    for b in range(B):
        for q0 in range(0, L, QCH):
            cb = cand[b, q0 : q0 + QCH]
            kc = Kn[b][:, cb.reshape(-1)].reshape(C9, QCH, NC)
            scores = np.einsum("cqj,cq->qj", kc, Qn[b][:, q0 : q0 + QCH],
                               dtype=np.float64)
            maxv = scores.max(axis=1, keepdims=True)
            kmask = np.where(scores == maxv, cb, 1 << 40)
            rarg[b, q0 : q0 + QCH] = kmask.min(axis=1)  # first occurrence on ties
            rstar[b, q0 : q0 + QCH] = maxv[:, 0].astype(np.float32)

    V_unf = _unfold_torch(V)
    T_unf = np.take_along_axis(V_unf, rarg[:, None, :], axis=2)
    T = (_fold_torch(T_unf, H, W) / 9.0).astype(np.float32)
    S = rstar.reshape(B, 1, H, W)

    if _return_results:
        return (S, T), res
    return (S, T)
